# revision 16
# baseline (speedup 1.0000x reference)
"""VQ codebook (DiscreteBottleneck) Trainium2 kernel.

Problem: slot_embeddings [64, 256, 1024] f32, codebook [4096, 1024] f32.
Returns (quantized_st [64,256,1024] f32, codes [64,256] i32,
         probs [64,256,4096] f32, vq_loss f32 scalar).

Strategy: data-parallel over batch across 8 NeuronCores (2048 flat rows
per core). Per core:
  score[n, c] = 2 * x_n . c_c - ||c_c||^2   (= -squared-distance + ||x||^2)
  codes = argmax_c score        (same as argmin of distance)
  probs = softmax(score)        (softmax is invariant to the per-row shift)
  quantized = codebook[codes]   (indirect-DMA row gather)
  quantized_st = x + (q - x);  vq_loss partial = sum((q - x)^2)

The 2048x4096x1024 contraction runs on the PE as a 3-pass split-bf16
matmul (x = xh + xl, c = ch + cl in bf16; xc ~= xh.ch + xh.cl + xl.ch).
Products of bf16 pairs are exact in the fp32 PSUM accumulator, so this
matches fp32-matmul argmin decisions (verified == fp64 argmin on the
problem's inputs) at 3 PE cycles/column instead of fp32's 4.
Host pre-computes the bf16 hi/lo splits of x and of 2*C^T, plus ||c||^2
replicated across partitions; host does the final loss reduction (the
"all-reduce").

Pipelining: block b+1's x DMAs + PE transposes are emitted right after
block b's matmuls (ahead of block b's softmax tail in engine priority);
each block's 8 hi (lo) transposes share one PSUM bank and drain with a
single copy.  Block 0 accumulates d-block-outer in two 4-c-tile waves so
its first matmuls only wait on the first codebook slice instead of the
whole 16.8MB load.
"""

import numpy as np

N_CORES = 8
BATCH, K, D = 64, 256, 1024
CB = 4096
N_TOTAL = BATCH * K          # 16384
N_CORE = N_TOTAL // N_CORES  # 2048
N_BLOCKS = N_CORE // 128     # 16
C_TILE = 512
N_CTILES = CB // C_TILE      # 8
N_DBLKS = D // 128           # 8
BETA = 0.25

_compiled = {}


def _build_bass():
    import concourse.bass as bass
    import concourse.bacc as bacc
    import concourse.mybir as mybir
    import concourse.tile as tile
    from concourse.masks import make_identity

    f32 = mybir.dt.float32
    bf16 = mybir.dt.bfloat16
    u32 = mybir.dt.uint32
    i32 = mybir.dt.int32
    Alu = mybir.AluOpType
    Act = mybir.ActivationFunctionType

    nc = bacc.Bacc("TRN2", target_bir_lowering=False, debug=False)
    x_in = nc.dram_tensor("x", [N_CORE, D], f32, kind="ExternalInput")
    xh_in = nc.dram_tensor("xh", [N_CORE, D], bf16, kind="ExternalInput")
    xl_in = nc.dram_tensor("xl", [N_CORE, D], bf16, kind="ExternalInput")
    cth_in = nc.dram_tensor("cth", [D, CB], bf16, kind="ExternalInput")  # hi(2*C^T)
    ctl_in = nc.dram_tensor("ctl", [D, CB], bf16, kind="ExternalInput")  # lo(2*C^T)
    sqc_in = nc.dram_tensor("sqc", [128, CB], f32, kind="ExternalInput")
    cb_in = nc.dram_tensor("cb", [CB, D], f32, kind="ExternalInput")
    probs_out = nc.dram_tensor("probs", [N_CORE, CB], f32, kind="ExternalOutput")
    qst_out = nc.dram_tensor("qst", [N_CORE, D], f32, kind="ExternalOutput")
    codes_out = nc.dram_tensor("codes", [N_CORE, 1], i32, kind="ExternalOutput")
    mse_out = nc.dram_tensor("msepart", [128, 1], f32, kind="ExternalOutput")

    cth_view = cth_in.rearrange("(t p) c -> t p c", p=128)
    ctl_view = ctl_in.rearrange("(t p) c -> t p c", p=128)

    with tile.TileContext(nc) as tc:
        with (
            tc.tile_pool(name="const", bufs=1) as const_pool,
            tc.tile_pool(name="ct", bufs=1) as ct_pool,
            tc.tile_pool(name="score", bufs=2) as score_pool,
            tc.tile_pool(name="x", bufs=2) as x_pool,
            tc.tile_pool(name="xhl", bufs=1) as xhl_pool,
            tc.tile_pool(name="xt", bufs=2) as xt_pool,
            tc.tile_pool(name="q", bufs=2) as q_pool,
            tc.tile_pool(name="small", bufs=2) as small_pool,
            tc.tile_pool(name="acc", bufs=1) as acc_pool,
            tc.tile_pool(name="psmm", bufs=6, space="PSUM") as psmm_pool,
            tc.tile_pool(name="pstr", bufs=2, space="PSUM") as pstr_pool,
        ):
            ident = const_pool.tile([128, 128], bf16, tag="ident")
            make_identity(nc, ident[:])

            def emit_xprep(nb):
                """DMA + transpose block nb's x rows; returns (xth, xtl, x)."""
                row0 = nb * 128
                xh_t = xhl_pool.tile([128, D], bf16, tag="xh", name=f"xh{nb}")
                nc.sync.dma_start(xh_t[:], xh_in[row0 : row0 + 128, :])
                xl_t = xhl_pool.tile([128, D], bf16, tag="xl", name=f"xl{nb}")
                nc.sync.dma_start(xl_t[:], xl_in[row0 : row0 + 128, :])

                xth_t = xt_pool.tile([128, D], bf16, tag="xth", name=f"xth{nb}")
                xtl_t = xt_pool.tile([128, D], bf16, tag="xtl", name=f"xtl{nb}")
                ps_hi = pstr_pool.tile([128, D], bf16, tag="pstr", name=f"psh{nb}")
                for db in range(N_DBLKS):
                    dsl = slice(db * 128, (db + 1) * 128)
                    nc.tensor.transpose(ps_hi[:, dsl], xh_t[:, dsl], ident[:])
                nc.scalar.copy(xth_t[:], ps_hi[:])
                ps_lo = pstr_pool.tile([128, D], bf16, tag="pstr", name=f"psl{nb}")
                for db in range(N_DBLKS):
                    dsl = slice(db * 128, (db + 1) * 128)
                    nc.tensor.transpose(ps_lo[:, dsl], xl_t[:, dsl], ident[:])
                nc.vector.tensor_copy(xtl_t[:], ps_lo[:])

                x_t = x_pool.tile([128, D], f32, tag="x", name=f"x{nb}")
                nc.scalar.dma_start(x_t[:], x_in[row0 : row0 + 128, :])
                return xth_t, xtl_t, x_t

            # block 0's x rows first in the DMA queue, then the codebook
            cur_prep = emit_xprep(0)

            ct_tiles = []  # [(hi, lo)] per d-block
            for t in range(N_DBLKS):
                cth_t = ct_pool.tile([128, CB], bf16, tag=f"cth{t}")
                nc.sync.dma_start(cth_t[:], cth_view[t])
                ctl_t = ct_pool.tile([128, CB], bf16, tag=f"ctl{t}")
                nc.sync.dma_start(ctl_t[:], ctl_view[t])
                ct_tiles.append((cth_t, ctl_t))

            sqc_sb = const_pool.tile([128, CB], f32, tag="sqc")
            nc.scalar.dma_start(sqc_sb[:], sqc_in[:])

            mse_cols = acc_pool.tile([128, N_BLOCKS], f32, tag="msecols")

            passes = lambda xth_t, xtl_t, dsl: (
                (xth_t[:, dsl], 0), (xth_t[:, dsl], 1), (xtl_t[:, dsl], 0)
            )

            for blk in range(N_BLOCKS):
                row0 = blk * 128
                xth_t, xtl_t, x_t = cur_prep

                score_t = score_pool.tile([128, CB], f32, tag="score",
                                          name=f"score{blk}")
                max8s = small_pool.tile([128, 8 * N_CTILES], f32, tag="max8s",
                                        name=f"max8s{blk}")

                # ---- matmuls ----
                pss = {}
                if blk == 0:
                    # d-block-outer waves (6 then 2 c-tiles) for the cold
                    # start: maximize PE work per arriving codebook slice
                    for wave, cis in enumerate((list(range(6)), [6, 7])):
                        for ci in cis:
                            ps_mm = psmm_pool.tile(
                                [128, C_TILE], f32, tag="psmm",
                                name=f"ps{blk}_{ci}",
                            )
                            pss[ci] = ps_mm
                        for db in range(N_DBLKS):
                            dsl = slice(db * 128, (db + 1) * 128)
                            for pi, (lhsT, hl) in enumerate(
                                passes(xth_t, xtl_t, dsl)
                            ):
                                rt = ct_tiles[db][hl]
                                for ci in cis:
                                    csl = slice(ci * C_TILE, (ci + 1) * C_TILE)
                                    nc.tensor.matmul(
                                        pss[ci][:], lhsT=lhsT, rhs=rt[:, csl],
                                        start=(db == 0 and pi == 0),
                                        stop=(db == N_DBLKS - 1 and pi == 2),
                                    )
                else:
                    for ci in range(N_CTILES):
                        csl = slice(ci * C_TILE, (ci + 1) * C_TILE)
                        ps_mm = psmm_pool.tile(
                            [128, C_TILE], f32, tag="psmm", name=f"ps{blk}_{ci}"
                        )
                        pss[ci] = ps_mm
                        k = 0
                        for db in range(N_DBLKS):
                            dsl = slice(db * 128, (db + 1) * 128)
                            for lhsT, hl in passes(xth_t, xtl_t, dsl):
                                nc.tensor.matmul(
                                    ps_mm[:], lhsT=lhsT,
                                    rhs=ct_tiles[db][hl][:, csl],
                                    start=(k == 0), stop=(k == 3 * N_DBLKS - 1),
                                )
                                k += 1

                # next block's transposes go ahead of this block's tail
                if blk + 1 < N_BLOCKS:
                    cur_prep = emit_xprep(blk + 1)

                # ---- psum drain + per-c-tile top-8 ----
                for ci in range(N_CTILES):
                    csl = slice(ci * C_TILE, (ci + 1) * C_TILE)
                    nc.vector.tensor_tensor(
                        out=score_t[:, csl], in0=pss[ci][:],
                        in1=sqc_sb[:, csl], op=Alu.subtract,
                    )
                    nc.vector.max(
                        max8s[:, ci * 8 : (ci + 1) * 8], score_t[:, csl]
                    )

                # ---- softmax / argmax tail ----
                max8 = small_pool.tile([128, 8], f32, tag="max8",
                                       name=f"max8_{blk}")
                idx8 = small_pool.tile([128, 8], u32, tag="idx8",
                                       name=f"idx8_{blk}")
                nc.vector.max(max8[:], max8s[:])
                nc.vector.max_index(idx8[:], max8[:], score_t[:])

                negmax = small_pool.tile([128, 1], f32, tag="negmax",
                                         name=f"negmax{blk}")
                nc.vector.tensor_scalar_mul(negmax[:], max8[:, 0:1], -1.0)

                sumexp = small_pool.tile([128, 1], f32, tag="sumexp",
                                         name=f"sumexp{blk}")
                nc.scalar.activation(
                    score_t[:], score_t[:], Act.Exp,
                    bias=negmax[:, 0:1], scale=1.0, accum_out=sumexp[:, 0:1],
                )
                rcp = small_pool.tile([128, 1], f32, tag="rcp", name=f"rcp{blk}")
                nc.vector.reciprocal(rcp[:], sumexp[:])
                # normalize + store in halves so the DMA overlaps the mul
                for h in range(2):
                    hsl = slice(h * (CB // 2), (h + 1) * (CB // 2))
                    nc.vector.tensor_scalar_mul(
                        score_t[:, hsl], score_t[:, hsl], rcp[:, 0:1]
                    )
                    nc.scalar.dma_start(
                        probs_out[row0 : row0 + 128, hsl], score_t[:, hsl]
                    )

                codes_t = small_pool.tile([128, 1], i32, tag="codes",
                                          name=f"codes{blk}")
                nc.vector.tensor_copy(codes_t[:], idx8[:, 0:1])
                nc.scalar.dma_start(codes_out[row0 : row0 + 128, :], codes_t[:])

                q_t = q_pool.tile([128, D], f32, tag="q", name=f"q{blk}")
                nc.gpsimd.indirect_dma_start(
                    out=q_t[:],
                    out_offset=None,
                    in_=cb_in[:, :],
                    in_offset=bass.IndirectOffsetOnAxis(ap=idx8[:, 0:1], axis=0),
                )
                # diff = q - x (in place in q); qst = x + diff (in place in x)
                nc.gpsimd.tensor_tensor(
                    out=q_t[:], in0=q_t[:], in1=x_t[:], op=Alu.subtract
                )
                nc.gpsimd.tensor_tensor(
                    out=x_t[:], in0=x_t[:], in1=q_t[:], op=Alu.add
                )
                nc.scalar.dma_start(qst_out[row0 : row0 + 128, :], x_t[:])
                nc.scalar.activation(
                    q_t[:], q_t[:], Act.Square,
                    accum_out=mse_cols[:, blk : blk + 1],
                )

            mse_fin = acc_pool.tile([128, 1], f32, tag="msefin")
            nc.vector.reduce_sum(mse_fin[:], mse_cols[:], axis=mybir.AxisListType.X)
            nc.sync.dma_start(mse_out[:], mse_fin[:])

    nc.compile()
    return nc


def _get_nc():
    if "nc" not in _compiled:
        _compiled["nc"] = _build_bass()
    return _compiled["nc"]


def kernel(slot_embeddings: np.ndarray, codebook: np.ndarray):
    import ml_dtypes
    from concourse.bass_utils import run_bass_kernel_spmd

    bf16 = ml_dtypes.bfloat16
    x = np.ascontiguousarray(slot_embeddings.reshape(N_TOTAL, D), dtype=np.float32)
    cb = np.ascontiguousarray(codebook, dtype=np.float32)

    xh = x.astype(bf16)
    xl = (x - xh.astype(np.float32)).astype(bf16)
    ct2 = np.ascontiguousarray(cb.T) * np.float32(2.0)
    cth = ct2.astype(bf16)
    ctl = (ct2 - cth.astype(np.float32)).astype(bf16)
    sqc = np.sum(cb.astype(np.float64) ** 2, axis=1).astype(np.float32)
    sqc_rep = np.ascontiguousarray(np.broadcast_to(sqc[None, :], (128, CB)))

    nc = _get_nc()
    in_maps = []
    for c in range(N_CORES):
        sl = slice(c * N_CORE, (c + 1) * N_CORE)
        in_maps.append(
            {
                "x": x[sl],
                "xh": xh[sl],
                "xl": xl[sl],
                "cth": cth,
                "ctl": ctl,
                "sqc": sqc_rep,
                "cb": cb,
            }
        )
    res = run_bass_kernel_spmd(nc, in_maps, core_ids=list(range(N_CORES)))
    _compiled["last_res"] = res

    probs = np.empty((N_TOTAL, CB), dtype=np.float32)
    qst = np.empty((N_TOTAL, D), dtype=np.float32)
    codes = np.empty((N_TOTAL,), dtype=np.int32)
    sum_sq = np.float32(0.0)
    for c, r in enumerate(res.results):
        sl = slice(c * N_CORE, (c + 1) * N_CORE)
        probs[sl] = r["probs"]
        qst[sl] = r["qst"]
        codes[sl] = r["codes"][:, 0]
        sum_sq = np.float32(sum_sq + np.float32(np.sum(r["msepart"], dtype=np.float64)))

    mse = np.float32(sum_sq / np.float32(N_TOTAL * D))
    vq_loss = np.float32(mse + np.float32(BETA) * mse)

    return (
        qst.reshape(BATCH, K, D),
        codes.reshape(BATCH, K),
        probs.reshape(BATCH, K, CB),
        vq_loss,
    )


# revision 17
# speedup vs baseline: 1.0612x; 1.0612x over previous
"""VQ codebook (DiscreteBottleneck) Trainium2 kernel.

Problem: slot_embeddings [64, 256, 1024] f32, codebook [4096, 1024] f32.
Returns (quantized_st [64,256,1024] f32, codes [64,256] i32,
         probs [64,256,4096] f32, vq_loss f32 scalar).

Strategy: data-parallel over batch across 8 NeuronCores (2048 flat rows
per core). Per core:
  score[n, c] = 2 * x_n . c_c - ||c_c||^2   (= -squared-distance + ||x||^2)
  codes = argmax_c score        (same as argmin of distance)
  probs = softmax(score)        (softmax is invariant to the per-row shift)
  quantized = codebook[codes]   (indirect-DMA row gather)
  quantized_st = x + (q - x);  vq_loss partial = sum((q - x)^2)

The 2048x4096x1024 contraction runs on the PE as a 3-pass split-bf16
matmul (x = xh + xl, c = ch + cl in bf16; xc ~= xh.ch + xh.cl + xl.ch).
Products of bf16 pairs are exact in the fp32 PSUM accumulator, so this
matches fp32-matmul argmin decisions (verified == fp64 argmin on the
problem's inputs) at 3 PE cycles/column instead of fp32's 4.
Host pre-computes the bf16 hi/lo splits of x and of 2*C^T, plus ||c||^2
replicated across partitions; host does the final loss reduction (the
"all-reduce").

Pipelining: block b+1's x DMAs + PE transposes are emitted right after
block b's matmuls (ahead of block b's softmax tail in engine priority);
each block's 8 hi (lo) transposes share one PSUM bank and drain with a
single copy.  Block 0 accumulates d-block-outer in two 4-c-tile waves so
its first matmuls only wait on the first codebook slice instead of the
whole 16.8MB load.
"""

import numpy as np

N_CORES = 8
BATCH, K, D = 64, 256, 1024
CB = 4096
N_TOTAL = BATCH * K          # 16384
N_CORE = N_TOTAL // N_CORES  # 2048
N_BLOCKS = N_CORE // 128     # 16
C_TILE = 512
N_CTILES = CB // C_TILE      # 8
N_DBLKS = D // 128           # 8
BETA = 0.25

_compiled = {}


def _build_bass():
    import concourse.bass as bass
    import concourse.bacc as bacc
    import concourse.mybir as mybir
    import concourse.tile as tile
    from concourse.masks import make_identity

    f32 = mybir.dt.float32
    bf16 = mybir.dt.bfloat16
    u32 = mybir.dt.uint32
    i32 = mybir.dt.int32
    Alu = mybir.AluOpType
    Act = mybir.ActivationFunctionType

    nc = bacc.Bacc("TRN2", target_bir_lowering=False, debug=False)
    x_in = nc.dram_tensor("x", [N_CORE, D], f32, kind="ExternalInput")
    # pre-transposed per-block x splits: [blk][p=d-in-dblk][db*128+n]
    xth_in = nc.dram_tensor("xth", [N_BLOCKS, 128, D], bf16, kind="ExternalInput")
    xtl_in = nc.dram_tensor("xtl", [N_BLOCKS, 128, D], bf16, kind="ExternalInput")
    cth_in = nc.dram_tensor("cth", [D, CB], bf16, kind="ExternalInput")  # hi(2*C^T)
    ctl_in = nc.dram_tensor("ctl", [D, CB], bf16, kind="ExternalInput")  # lo(2*C^T)
    sqc_in = nc.dram_tensor("sqc", [128, CB], f32, kind="ExternalInput")
    cb_in = nc.dram_tensor("cb", [CB, D], f32, kind="ExternalInput")
    probs_out = nc.dram_tensor("probs", [N_CORE, CB], f32, kind="ExternalOutput")
    qst_out = nc.dram_tensor("qst", [N_CORE, D], f32, kind="ExternalOutput")
    codes_out = nc.dram_tensor("codes", [N_CORE, 1], i32, kind="ExternalOutput")
    mse_out = nc.dram_tensor("msepart", [128, 1], f32, kind="ExternalOutput")

    cth_view = cth_in.rearrange("(t p) c -> t p c", p=128)
    ctl_view = ctl_in.rearrange("(t p) c -> t p c", p=128)

    with tile.TileContext(nc) as tc:
        with (
            tc.tile_pool(name="const", bufs=1) as const_pool,
            tc.tile_pool(name="ct", bufs=1) as ct_pool,
            tc.tile_pool(name="score", bufs=2) as score_pool,
            tc.tile_pool(name="x", bufs=2) as x_pool,
            tc.tile_pool(name="xt", bufs=2) as xt_pool,
            tc.tile_pool(name="q", bufs=2) as q_pool,
            tc.tile_pool(name="small", bufs=2) as small_pool,
            tc.tile_pool(name="acc", bufs=1) as acc_pool,
            tc.tile_pool(name="psmm", bufs=8, space="PSUM") as psmm_pool,
        ):
            def emit_xprep(nb):
                """DMA block nb's pre-transposed x splits + x rows."""
                row0 = nb * 128
                xth_t = xt_pool.tile([128, D], bf16, tag="xth", name=f"xth{nb}")
                nc.sync.dma_start(xth_t[:], xth_in[nb])
                xtl_t = xt_pool.tile([128, D], bf16, tag="xtl", name=f"xtl{nb}")
                nc.sync.dma_start(xtl_t[:], xtl_in[nb])
                x_t = x_pool.tile([128, D], f32, tag="x", name=f"x{nb}")
                nc.scalar.dma_start(x_t[:], x_in[row0 : row0 + 128, :])
                return xth_t, xtl_t, x_t

            # block 0's x rows first in the DMA queue, then the codebook
            cur_prep = emit_xprep(0)

            ct_tiles = []  # [(hi, lo)] per d-block
            for t in range(N_DBLKS):
                cth_t = ct_pool.tile([128, CB], bf16, tag=f"cth{t}")
                nc.sync.dma_start(cth_t[:], cth_view[t])
                ctl_t = ct_pool.tile([128, CB], bf16, tag=f"ctl{t}")
                nc.sync.dma_start(ctl_t[:], ctl_view[t])
                ct_tiles.append((cth_t, ctl_t))

            sqc_sb = const_pool.tile([128, CB], f32, tag="sqc")
            nc.scalar.dma_start(sqc_sb[:], sqc_in[:])

            mse_cols = acc_pool.tile([128, N_BLOCKS], f32, tag="msecols")

            passes = lambda xth_t, xtl_t, dsl: (
                (xth_t[:, dsl], 0), (xth_t[:, dsl], 1), (xtl_t[:, dsl], 0)
            )

            for blk in range(N_BLOCKS):
                row0 = blk * 128
                xth_t, xtl_t, x_t = cur_prep

                score_t = score_pool.tile([128, CB], f32, tag="score",
                                          name=f"score{blk}")
                max8s = small_pool.tile([128, 8 * N_CTILES], f32, tag="max8s",
                                        name=f"max8s{blk}")

                # ---- matmuls ----
                pss = {}
                if blk == 0:
                    # one d-block-outer wave over all 8 c-tiles (cold
                    # start): each db step only needs that db's ct slice
                    for wave, cis in enumerate((list(range(N_CTILES)),)):
                        for ci in cis:
                            ps_mm = psmm_pool.tile(
                                [128, C_TILE], f32, tag="psmm",
                                name=f"ps{blk}_{ci}",
                            )
                            pss[ci] = ps_mm
                        for db in range(N_DBLKS):
                            dsl = slice(db * 128, (db + 1) * 128)
                            for pi, (lhsT, hl) in enumerate(
                                passes(xth_t, xtl_t, dsl)
                            ):
                                rt = ct_tiles[db][hl]
                                for ci in cis:
                                    csl = slice(ci * C_TILE, (ci + 1) * C_TILE)
                                    nc.tensor.matmul(
                                        pss[ci][:], lhsT=lhsT, rhs=rt[:, csl],
                                        start=(db == 0 and pi == 0),
                                        stop=(db == N_DBLKS - 1 and pi == 2),
                                    )
                else:
                    for ci in range(N_CTILES):
                        csl = slice(ci * C_TILE, (ci + 1) * C_TILE)
                        ps_mm = psmm_pool.tile(
                            [128, C_TILE], f32, tag="psmm", name=f"ps{blk}_{ci}"
                        )
                        pss[ci] = ps_mm
                        k = 0
                        for db in range(N_DBLKS):
                            dsl = slice(db * 128, (db + 1) * 128)
                            for lhsT, hl in passes(xth_t, xtl_t, dsl):
                                nc.tensor.matmul(
                                    ps_mm[:], lhsT=lhsT,
                                    rhs=ct_tiles[db][hl][:, csl],
                                    start=(k == 0), stop=(k == 3 * N_DBLKS - 1),
                                )
                                k += 1

                # next block's transposes go ahead of this block's tail
                if blk + 1 < N_BLOCKS:
                    cur_prep = emit_xprep(blk + 1)

                # ---- psum drain + per-c-tile top-8 ----
                for ci in range(N_CTILES):
                    csl = slice(ci * C_TILE, (ci + 1) * C_TILE)
                    nc.vector.tensor_tensor(
                        out=score_t[:, csl], in0=pss[ci][:],
                        in1=sqc_sb[:, csl], op=Alu.subtract,
                    )
                    nc.vector.max(
                        max8s[:, ci * 8 : (ci + 1) * 8], score_t[:, csl]
                    )

                # ---- softmax / argmax tail ----
                max8 = small_pool.tile([128, 8], f32, tag="max8",
                                       name=f"max8_{blk}")
                idx8 = small_pool.tile([128, 8], u32, tag="idx8",
                                       name=f"idx8_{blk}")
                nc.vector.max(max8[:], max8s[:])
                nc.vector.max_index(idx8[:], max8[:], score_t[:])

                negmax = small_pool.tile([128, 1], f32, tag="negmax",
                                         name=f"negmax{blk}")
                nc.vector.tensor_scalar_mul(negmax[:], max8[:, 0:1], -1.0)

                sumexp = small_pool.tile([128, 1], f32, tag="sumexp",
                                         name=f"sumexp{blk}")
                nc.scalar.activation(
                    score_t[:], score_t[:], Act.Exp,
                    bias=negmax[:, 0:1], scale=1.0, accum_out=sumexp[:, 0:1],
                )
                rcp = small_pool.tile([128, 1], f32, tag="rcp", name=f"rcp{blk}")
                nc.vector.reciprocal(rcp[:], sumexp[:])
                # normalize + store in halves so the DMA overlaps the mul
                for h in range(2):
                    hsl = slice(h * (CB // 2), (h + 1) * (CB // 2))
                    nc.vector.tensor_scalar_mul(
                        score_t[:, hsl], score_t[:, hsl], rcp[:, 0:1]
                    )
                    nc.scalar.dma_start(
                        probs_out[row0 : row0 + 128, hsl], score_t[:, hsl]
                    )

                codes_t = small_pool.tile([128, 1], i32, tag="codes",
                                          name=f"codes{blk}")
                nc.vector.tensor_copy(codes_t[:], idx8[:, 0:1])
                nc.scalar.dma_start(codes_out[row0 : row0 + 128, :], codes_t[:])

                q_t = q_pool.tile([128, D], f32, tag="q", name=f"q{blk}")
                nc.gpsimd.indirect_dma_start(
                    out=q_t[:],
                    out_offset=None,
                    in_=cb_in[:, :],
                    in_offset=bass.IndirectOffsetOnAxis(ap=idx8[:, 0:1], axis=0),
                )
                # diff = q - x (in place in q); qst = x + diff (in place in x)
                nc.gpsimd.tensor_tensor(
                    out=q_t[:], in0=q_t[:], in1=x_t[:], op=Alu.subtract
                )
                nc.gpsimd.tensor_tensor(
                    out=x_t[:], in0=x_t[:], in1=q_t[:], op=Alu.add
                )
                nc.scalar.dma_start(qst_out[row0 : row0 + 128, :], x_t[:])
                nc.scalar.activation(
                    q_t[:], q_t[:], Act.Square,
                    accum_out=mse_cols[:, blk : blk + 1],
                )

            mse_fin = acc_pool.tile([128, 1], f32, tag="msefin")
            nc.vector.reduce_sum(mse_fin[:], mse_cols[:], axis=mybir.AxisListType.X)
            nc.sync.dma_start(mse_out[:], mse_fin[:])

    nc.compile()
    return nc


def _get_nc():
    if "nc" not in _compiled:
        _compiled["nc"] = _build_bass()
    return _compiled["nc"]


def kernel(slot_embeddings: np.ndarray, codebook: np.ndarray):
    import ml_dtypes
    from concourse.bass_utils import run_bass_kernel_spmd

    bf16 = ml_dtypes.bfloat16
    x = np.ascontiguousarray(slot_embeddings.reshape(N_TOTAL, D), dtype=np.float32)
    cb = np.ascontiguousarray(codebook, dtype=np.float32)

    xh = x.astype(bf16)
    xl = (x - xh.astype(np.float32)).astype(bf16)
    # pre-transposed per-core block layout [blk, p, db*128+n]:
    #   xt[core][blk, p, db, n] = xh.T[db*128+p, core*2048 + blk*128+n]
    def to_blocks(a):  # a: [N_TOTAL, D] bf16 -> [N_CORES, N_BLOCKS, 128, D]
        t = np.ascontiguousarray(a.T)  # [D, N_TOTAL]
        t = t.reshape(N_DBLKS, 128, N_CORES, N_BLOCKS, 128)
        return np.ascontiguousarray(t.transpose(2, 3, 1, 0, 4)).reshape(
            N_CORES, N_BLOCKS, 128, D
        )

    xth_b = to_blocks(xh)
    xtl_b = to_blocks(xl)
    ct2 = np.ascontiguousarray(cb.T) * np.float32(2.0)
    cth = ct2.astype(bf16)
    ctl = (ct2 - cth.astype(np.float32)).astype(bf16)
    sqc = np.sum(cb.astype(np.float64) ** 2, axis=1).astype(np.float32)
    sqc_rep = np.ascontiguousarray(np.broadcast_to(sqc[None, :], (128, CB)))

    nc = _get_nc()
    in_maps = []
    for c in range(N_CORES):
        sl = slice(c * N_CORE, (c + 1) * N_CORE)
        in_maps.append(
            {
                "x": x[sl],
                "xth": xth_b[c],
                "xtl": xtl_b[c],
                "cth": cth,
                "ctl": ctl,
                "sqc": sqc_rep,
                "cb": cb,
            }
        )
    res = run_bass_kernel_spmd(nc, in_maps, core_ids=list(range(N_CORES)))
    _compiled["last_res"] = res

    probs = np.empty((N_TOTAL, CB), dtype=np.float32)
    qst = np.empty((N_TOTAL, D), dtype=np.float32)
    codes = np.empty((N_TOTAL,), dtype=np.int32)
    sum_sq = np.float32(0.0)
    for c, r in enumerate(res.results):
        sl = slice(c * N_CORE, (c + 1) * N_CORE)
        probs[sl] = r["probs"]
        qst[sl] = r["qst"]
        codes[sl] = r["codes"][:, 0]
        sum_sq = np.float32(sum_sq + np.float32(np.sum(r["msepart"], dtype=np.float64)))

    mse = np.float32(sum_sq / np.float32(N_TOTAL * D))
    vq_loss = np.float32(mse + np.float32(BETA) * mse)

    return (
        qst.reshape(BATCH, K, D),
        codes.reshape(BATCH, K),
        probs.reshape(BATCH, K, CB),
        vq_loss,
    )


# revision 18
# speedup vs baseline: 1.0723x; 1.0105x over previous
"""VQ codebook (DiscreteBottleneck) Trainium2 kernel.

Problem: slot_embeddings [64, 256, 1024] f32, codebook [4096, 1024] f32.
Returns (quantized_st [64,256,1024] f32, codes [64,256] i32,
         probs [64,256,4096] f32, vq_loss f32 scalar).

Strategy: data-parallel over batch across 8 NeuronCores (2048 flat rows
per core). Per core:
  score[n, c] = 2 * x_n . c_c - ||c_c||^2   (= -squared-distance + ||x||^2)
  codes = argmax_c score        (same as argmin of distance)
  probs = softmax(score)        (softmax is invariant to the per-row shift)
  quantized = codebook[codes]   (indirect-DMA row gather)
  quantized_st = x + (q - x);  vq_loss partial = sum((q - x)^2)

The 2048x4096x1024 contraction runs on the PE as a 3-pass split-bf16
matmul (x = xh + xl, c = ch + cl in bf16; xc ~= xh.ch + xh.cl + xl.ch).
Products of bf16 pairs are exact in the fp32 PSUM accumulator, so this
matches fp32-matmul argmin decisions (verified == fp64 argmin on the
problem's inputs) at 3 PE cycles/column instead of fp32's 4.
Host pre-computes the bf16 hi/lo splits of x and of 2*C^T, plus ||c||^2
replicated across partitions; host does the final loss reduction (the
"all-reduce").

Pipelining: block b+1's x DMAs + PE transposes are emitted right after
block b's matmuls (ahead of block b's softmax tail in engine priority);
each block's 8 hi (lo) transposes share one PSUM bank and drain with a
single copy.  Block 0 accumulates d-block-outer in two 4-c-tile waves so
its first matmuls only wait on the first codebook slice instead of the
whole 16.8MB load.
"""

import numpy as np

N_CORES = 8
BATCH, K, D = 64, 256, 1024
CB = 4096
N_TOTAL = BATCH * K          # 16384
N_CORE = N_TOTAL // N_CORES  # 2048
N_BLOCKS = N_CORE // 128     # 16
C_TILE = 512
N_CTILES = CB // C_TILE      # 8
N_DBLKS = D // 128           # 8
BETA = 0.25

_compiled = {}


def _build_bass():
    import concourse.bass as bass
    import concourse.bacc as bacc
    import concourse.mybir as mybir
    import concourse.tile as tile
    from concourse.masks import make_identity

    f32 = mybir.dt.float32
    bf16 = mybir.dt.bfloat16
    u32 = mybir.dt.uint32
    i32 = mybir.dt.int32
    Alu = mybir.AluOpType
    Act = mybir.ActivationFunctionType

    nc = bacc.Bacc("TRN2", target_bir_lowering=False, debug=False)
    x_in = nc.dram_tensor("x", [N_CORE, D], f32, kind="ExternalInput")
    # pre-transposed per-block x splits: [blk][p=d-in-dblk][db*128+n]
    xth_in = nc.dram_tensor("xth", [N_BLOCKS, 128, D], bf16, kind="ExternalInput")
    xtl_in = nc.dram_tensor("xtl", [N_BLOCKS, 128, D], bf16, kind="ExternalInput")
    cth_in = nc.dram_tensor("cth", [D, CB], bf16, kind="ExternalInput")  # hi(2*C^T)
    ctl_in = nc.dram_tensor("ctl", [D, CB], bf16, kind="ExternalInput")  # lo(2*C^T)
    sqc_in = nc.dram_tensor("sqc", [128, CB], f32, kind="ExternalInput")
    cb_in = nc.dram_tensor("cb", [CB, D], f32, kind="ExternalInput")
    probs_out = nc.dram_tensor("probs", [N_CORE, CB], f32, kind="ExternalOutput")
    qst_out = nc.dram_tensor("qst", [N_CORE, D], f32, kind="ExternalOutput")
    codes_out = nc.dram_tensor("codes", [N_CORE, 1], i32, kind="ExternalOutput")
    mse_out = nc.dram_tensor("msepart", [128, N_BLOCKS], f32, kind="ExternalOutput")

    cth_view = cth_in.rearrange("(t p) c -> t p c", p=128)
    ctl_view = ctl_in.rearrange("(t p) c -> t p c", p=128)

    with tile.TileContext(nc) as tc:
        with (
            tc.tile_pool(name="const", bufs=1) as const_pool,
            tc.tile_pool(name="ct", bufs=1) as ct_pool,
            tc.tile_pool(name="score", bufs=2) as score_pool,
            tc.tile_pool(name="x", bufs=2) as x_pool,
            tc.tile_pool(name="xt", bufs=2) as xt_pool,
            tc.tile_pool(name="q", bufs=2) as q_pool,
            tc.tile_pool(name="small", bufs=2) as small_pool,
            tc.tile_pool(name="acc", bufs=1) as acc_pool,
            tc.tile_pool(name="psmm", bufs=8, space="PSUM") as psmm_pool,
        ):
            def emit_xprep(nb):
                """DMA block nb's pre-transposed x splits + x rows."""
                row0 = nb * 128
                xth_t = xt_pool.tile([128, D], bf16, tag="xth", name=f"xth{nb}")
                nc.sync.dma_start(xth_t[:], xth_in[nb])
                xtl_t = xt_pool.tile([128, D], bf16, tag="xtl", name=f"xtl{nb}")
                nc.sync.dma_start(xtl_t[:], xtl_in[nb])
                x_t = x_pool.tile([128, D], f32, tag="x", name=f"x{nb}")
                nc.scalar.dma_start(x_t[:], x_in[row0 : row0 + 128, :])
                return xth_t, xtl_t, x_t

            # block 0's x rows first in the DMA queue, then the codebook
            cur_prep = emit_xprep(0)

            ct_tiles = []  # [(hi, lo)] per d-block
            for t in range(N_DBLKS):
                cth_t = ct_pool.tile([128, CB], bf16, tag=f"cth{t}")
                nc.sync.dma_start(cth_t[:], cth_view[t])
                ctl_t = ct_pool.tile([128, CB], bf16, tag=f"ctl{t}")
                nc.sync.dma_start(ctl_t[:], ctl_view[t])
                ct_tiles.append((cth_t, ctl_t))

            sqc_sb = const_pool.tile([128, CB], f32, tag="sqc")
            nc.scalar.dma_start(sqc_sb[:], sqc_in[:])

            mse_cols = acc_pool.tile([128, N_BLOCKS], f32, tag="msecols")

            passes = lambda xth_t, xtl_t, dsl: (
                (xth_t[:, dsl], 0), (xth_t[:, dsl], 1), (xtl_t[:, dsl], 0)
            )

            for blk in range(N_BLOCKS):
                row0 = blk * 128
                xth_t, xtl_t, x_t = cur_prep

                score_t = score_pool.tile([128, CB], f32, tag="score",
                                          name=f"score{blk}")
                max8s = small_pool.tile([128, 8 * N_CTILES], f32, tag="max8s",
                                        name=f"max8s{blk}")

                # ---- matmuls ----
                pss = {}
                if blk == 0:
                    # one d-block-outer wave over all 8 c-tiles (cold
                    # start): each db step only needs that db's ct slice
                    for wave, cis in enumerate((list(range(N_CTILES)),)):
                        for ci in cis:
                            ps_mm = psmm_pool.tile(
                                [128, C_TILE], f32, tag="psmm",
                                name=f"ps{blk}_{ci}",
                            )
                            pss[ci] = ps_mm
                        for db in range(N_DBLKS):
                            dsl = slice(db * 128, (db + 1) * 128)
                            for pi, (lhsT, hl) in enumerate(
                                passes(xth_t, xtl_t, dsl)
                            ):
                                rt = ct_tiles[db][hl]
                                for ci in cis:
                                    csl = slice(ci * C_TILE, (ci + 1) * C_TILE)
                                    nc.tensor.matmul(
                                        pss[ci][:], lhsT=lhsT, rhs=rt[:, csl],
                                        start=(db == 0 and pi == 0),
                                        stop=(db == N_DBLKS - 1 and pi == 2),
                                    )
                else:
                    for ci in range(N_CTILES):
                        csl = slice(ci * C_TILE, (ci + 1) * C_TILE)
                        ps_mm = psmm_pool.tile(
                            [128, C_TILE], f32, tag="psmm", name=f"ps{blk}_{ci}"
                        )
                        pss[ci] = ps_mm
                        k = 0
                        for db in range(N_DBLKS):
                            dsl = slice(db * 128, (db + 1) * 128)
                            for lhsT, hl in passes(xth_t, xtl_t, dsl):
                                nc.tensor.matmul(
                                    ps_mm[:], lhsT=lhsT,
                                    rhs=ct_tiles[db][hl][:, csl],
                                    start=(k == 0), stop=(k == 3 * N_DBLKS - 1),
                                )
                                k += 1

                # next block's transposes go ahead of this block's tail
                if blk + 1 < N_BLOCKS:
                    cur_prep = emit_xprep(blk + 1)

                # ---- psum drain + per-c-tile top-8 ----
                for ci in range(N_CTILES):
                    csl = slice(ci * C_TILE, (ci + 1) * C_TILE)
                    nc.vector.tensor_tensor(
                        out=score_t[:, csl], in0=pss[ci][:],
                        in1=sqc_sb[:, csl], op=Alu.subtract,
                    )
                    nc.vector.max(
                        max8s[:, ci * 8 : (ci + 1) * 8], score_t[:, csl]
                    )

                # ---- softmax / argmax tail ----
                max8 = small_pool.tile([128, 8], f32, tag="max8",
                                       name=f"max8_{blk}")
                idx8 = small_pool.tile([128, 8], u32, tag="idx8",
                                       name=f"idx8_{blk}")
                nc.vector.max(max8[:], max8s[:])
                nc.vector.max_index(idx8[:], max8[:], score_t[:])

                negmax = small_pool.tile([128, 1], f32, tag="negmax",
                                         name=f"negmax{blk}")
                nc.vector.tensor_scalar_mul(negmax[:], max8[:, 0:1], -1.0)

                sumexp = small_pool.tile([128, 1], f32, tag="sumexp",
                                         name=f"sumexp{blk}")
                nc.scalar.activation(
                    score_t[:], score_t[:], Act.Exp,
                    bias=negmax[:, 0:1], scale=1.0, accum_out=sumexp[:, 0:1],
                )
                rcp = small_pool.tile([128, 1], f32, tag="rcp", name=f"rcp{blk}")
                nc.vector.reciprocal(rcp[:], sumexp[:])
                # normalize + store in halves so the DMA overlaps the mul
                for h in range(2):
                    hsl = slice(h * (CB // 2), (h + 1) * (CB // 2))
                    nc.vector.tensor_scalar_mul(
                        score_t[:, hsl], score_t[:, hsl], rcp[:, 0:1]
                    )
                    nc.scalar.dma_start(
                        probs_out[row0 : row0 + 128, hsl], score_t[:, hsl]
                    )

                codes_t = small_pool.tile([128, 1], i32, tag="codes",
                                          name=f"codes{blk}")
                nc.vector.tensor_copy(codes_t[:], idx8[:, 0:1])
                nc.scalar.dma_start(codes_out[row0 : row0 + 128, :], codes_t[:])

                q_t = q_pool.tile([128, D], f32, tag="q", name=f"q{blk}")
                nc.gpsimd.indirect_dma_start(
                    out=q_t[:],
                    out_offset=None,
                    in_=cb_in[:, :],
                    in_offset=bass.IndirectOffsetOnAxis(ap=idx8[:, 0:1], axis=0),
                )
                # diff = q - x (in place in q); qst = x + diff (in place in x)
                nc.gpsimd.tensor_tensor(
                    out=q_t[:], in0=q_t[:], in1=x_t[:], op=Alu.subtract
                )
                nc.gpsimd.tensor_tensor(
                    out=x_t[:], in0=x_t[:], in1=q_t[:], op=Alu.add
                )
                nc.scalar.dma_start(qst_out[row0 : row0 + 128, :], x_t[:])
                nc.scalar.activation(
                    q_t[:], q_t[:], Act.Square,
                    accum_out=mse_cols[:, blk : blk + 1],
                )

            nc.sync.dma_start(mse_out[:], mse_cols[:])

    nc.compile()
    return nc


def _get_nc():
    if "nc" not in _compiled:
        _compiled["nc"] = _build_bass()
    return _compiled["nc"]


def kernel(slot_embeddings: np.ndarray, codebook: np.ndarray):
    import ml_dtypes
    from concourse.bass_utils import run_bass_kernel_spmd

    bf16 = ml_dtypes.bfloat16
    x = np.ascontiguousarray(slot_embeddings.reshape(N_TOTAL, D), dtype=np.float32)
    cb = np.ascontiguousarray(codebook, dtype=np.float32)

    xh = x.astype(bf16)
    xl = (x - xh.astype(np.float32)).astype(bf16)
    # pre-transposed per-core block layout [blk, p, db*128+n]:
    #   xt[core][blk, p, db, n] = xh.T[db*128+p, core*2048 + blk*128+n]
    def to_blocks(a):  # a: [N_TOTAL, D] bf16 -> [N_CORES, N_BLOCKS, 128, D]
        t = np.ascontiguousarray(a.T)  # [D, N_TOTAL]
        t = t.reshape(N_DBLKS, 128, N_CORES, N_BLOCKS, 128)
        return np.ascontiguousarray(t.transpose(2, 3, 1, 0, 4)).reshape(
            N_CORES, N_BLOCKS, 128, D
        )

    xth_b = to_blocks(xh)
    xtl_b = to_blocks(xl)
    ct2 = np.ascontiguousarray(cb.T) * np.float32(2.0)
    cth = ct2.astype(bf16)
    ctl = (ct2 - cth.astype(np.float32)).astype(bf16)
    sqc = np.sum(cb.astype(np.float64) ** 2, axis=1).astype(np.float32)
    sqc_rep = np.ascontiguousarray(np.broadcast_to(sqc[None, :], (128, CB)))

    nc = _get_nc()
    in_maps = []
    for c in range(N_CORES):
        sl = slice(c * N_CORE, (c + 1) * N_CORE)
        in_maps.append(
            {
                "x": x[sl],
                "xth": xth_b[c],
                "xtl": xtl_b[c],
                "cth": cth,
                "ctl": ctl,
                "sqc": sqc_rep,
                "cb": cb,
            }
        )
    res = run_bass_kernel_spmd(nc, in_maps, core_ids=list(range(N_CORES)))
    _compiled["last_res"] = res

    probs = np.empty((N_TOTAL, CB), dtype=np.float32)
    qst = np.empty((N_TOTAL, D), dtype=np.float32)
    codes = np.empty((N_TOTAL,), dtype=np.int32)
    sum_sq = np.float32(0.0)
    for c, r in enumerate(res.results):
        sl = slice(c * N_CORE, (c + 1) * N_CORE)
        probs[sl] = r["probs"]
        qst[sl] = r["qst"]
        codes[sl] = r["codes"][:, 0]
        sum_sq = np.float32(sum_sq + np.float32(np.sum(r["msepart"], dtype=np.float64)))

    mse = np.float32(sum_sq / np.float32(N_TOTAL * D))
    vq_loss = np.float32(mse + np.float32(BETA) * mse)

    return (
        qst.reshape(BATCH, K, D),
        codes.reshape(BATCH, K),
        probs.reshape(BATCH, K, CB),
        vq_loss,
    )


# revision 19
# speedup vs baseline: 1.0842x; 1.0110x over previous
"""VQ codebook (DiscreteBottleneck) Trainium2 kernel.

Problem: slot_embeddings [64, 256, 1024] f32, codebook [4096, 1024] f32.
Returns (quantized_st [64,256,1024] f32, codes [64,256] i32,
         probs [64,256,4096] f32, vq_loss f32 scalar).

Strategy: data-parallel over batch across 8 NeuronCores (2048 flat rows
per core). Per core:
  score[n, c] = 2 * x_n . c_c - ||c_c||^2   (= -squared-distance + ||x||^2)
  codes = argmax_c score        (same as argmin of distance)
  probs = softmax(score)        (softmax is invariant to the per-row shift)
  quantized = codebook[codes]   (indirect-DMA row gather)
  quantized_st = x + (q - x);  vq_loss partial = sum((q - x)^2)

The 2048x4096x1024 contraction runs on the PE as a 3-pass split-bf16
matmul (x = xh + xl, c = ch + cl in bf16; xc ~= xh.ch + xh.cl + xl.ch).
Products of bf16 pairs are exact in the fp32 PSUM accumulator, so this
matches fp32-matmul argmin decisions (verified == fp64 argmin on the
problem's inputs) at 3 PE cycles/column instead of fp32's 4.
Host pre-computes the bf16 hi/lo splits of x and of 2*C^T, plus ||c||^2
replicated across partitions; host does the final loss reduction (the
"all-reduce").

The host also pre-transposes the x splits into a DMA-friendly per-block
layout ([blk, p=d-within-dblock, db*128+n]), so the device does no
transposes at all and all 8 PSUM banks go to matmul accumulation.
Pipelining: block b+1's x DMAs are emitted right after block b's matmuls
(ahead of block b's softmax tail in engine priority).  Block 0
accumulates d-block-outer across all 8 c-tiles so its first matmuls only
wait on the first codebook slice instead of the whole 16.8MB load.
"""

import numpy as np

N_CORES = 8
BATCH, K, D = 64, 256, 1024
CB = 4096
N_TOTAL = BATCH * K          # 16384
N_CORE = N_TOTAL // N_CORES  # 2048
N_BLOCKS = N_CORE // 128     # 16
C_TILE = 512
N_CTILES = CB // C_TILE      # 8
N_DBLKS = D // 128           # 8
BETA = 0.25

_compiled = {}


def _build_bass():
    import concourse.bass as bass
    import concourse.bacc as bacc
    import concourse.mybir as mybir
    import concourse.tile as tile
    from concourse.masks import make_identity

    f32 = mybir.dt.float32
    bf16 = mybir.dt.bfloat16
    u32 = mybir.dt.uint32
    i32 = mybir.dt.int32
    Alu = mybir.AluOpType
    Act = mybir.ActivationFunctionType

    nc = bacc.Bacc("TRN2", target_bir_lowering=False, debug=False)
    x_in = nc.dram_tensor("x", [N_CORE, D], f32, kind="ExternalInput")
    # pre-transposed per-block x splits: [blk][p=d-in-dblk][db*128+n]
    xth_in = nc.dram_tensor("xth", [N_BLOCKS, 128, D], bf16, kind="ExternalInput")
    xtl_in = nc.dram_tensor("xtl", [N_BLOCKS, 128, D], bf16, kind="ExternalInput")
    cth_in = nc.dram_tensor("cth", [D, CB], bf16, kind="ExternalInput")  # hi(2*C^T)
    ctl_in = nc.dram_tensor("ctl", [D, CB], bf16, kind="ExternalInput")  # lo(2*C^T)
    sqc_in = nc.dram_tensor("sqc", [128, CB], f32, kind="ExternalInput")
    cb_in = nc.dram_tensor("cb", [CB, D], f32, kind="ExternalInput")
    probs_out = nc.dram_tensor("probs", [N_CORE, CB], f32, kind="ExternalOutput")
    qst_out = nc.dram_tensor("qst", [N_CORE, D], f32, kind="ExternalOutput")
    codes_out = nc.dram_tensor("codes", [N_CORE, 1], i32, kind="ExternalOutput")
    mse_out = nc.dram_tensor("msepart", [128, N_BLOCKS], f32, kind="ExternalOutput")

    cth_view = cth_in.rearrange("(t p) c -> t p c", p=128)
    ctl_view = ctl_in.rearrange("(t p) c -> t p c", p=128)

    with tile.TileContext(nc) as tc:
        with (
            tc.tile_pool(name="const", bufs=1) as const_pool,
            tc.tile_pool(name="ct", bufs=1) as ct_pool,
            tc.tile_pool(name="score", bufs=2) as score_pool,
            tc.tile_pool(name="x", bufs=2) as x_pool,
            tc.tile_pool(name="xt", bufs=2) as xt_pool,
            tc.tile_pool(name="q", bufs=2) as q_pool,
            tc.tile_pool(name="small", bufs=2) as small_pool,
            tc.tile_pool(name="acc", bufs=1) as acc_pool,
            tc.tile_pool(name="psmm", bufs=8, space="PSUM") as psmm_pool,
        ):
            def emit_xprep(nb):
                """DMA block nb's pre-transposed x splits + x rows."""
                row0 = nb * 128
                xth_t = xt_pool.tile([128, D], bf16, tag="xth", name=f"xth{nb}")
                nc.sync.dma_start(xth_t[:], xth_in[nb])
                xtl_t = xt_pool.tile([128, D], bf16, tag="xtl", name=f"xtl{nb}")
                nc.sync.dma_start(xtl_t[:], xtl_in[nb])
                x_t = x_pool.tile([128, D], f32, tag="x", name=f"x{nb}")
                nc.scalar.dma_start(x_t[:], x_in[row0 : row0 + 128, :])
                return xth_t, xtl_t, x_t

            # block 0's x rows first in the DMA queue, then the codebook
            cur_prep = emit_xprep(0)

            ct_tiles = []  # [(hi, lo)] per d-block
            for t in range(N_DBLKS):
                cth_t = ct_pool.tile([128, CB], bf16, tag=f"cth{t}")
                nc.sync.dma_start(cth_t[:], cth_view[t])
                ctl_t = ct_pool.tile([128, CB], bf16, tag=f"ctl{t}")
                nc.sync.dma_start(ctl_t[:], ctl_view[t])
                ct_tiles.append((cth_t, ctl_t))

            sqc_sb = const_pool.tile([128, CB], f32, tag="sqc")
            nc.scalar.dma_start(sqc_sb[:], sqc_in[:])

            mse_cols = acc_pool.tile([128, N_BLOCKS], f32, tag="msecols")

            passes = lambda xth_t, xtl_t, dsl: (
                (xth_t[:, dsl], 0), (xth_t[:, dsl], 1), (xtl_t[:, dsl], 0)
            )

            for blk in range(N_BLOCKS):
                row0 = blk * 128
                xth_t, xtl_t, x_t = cur_prep

                score_t = score_pool.tile([128, CB], f32, tag="score",
                                          name=f"score{blk}")
                max8s = small_pool.tile([128, 8 * N_CTILES], f32, tag="max8s",
                                        name=f"max8s{blk}")

                # ---- matmuls ----
                pss = {}
                if blk == 0:
                    # one d-block-outer wave over all 8 c-tiles (cold
                    # start): each db step only needs that db's ct slice
                    for wave, cis in enumerate((list(range(N_CTILES)),)):
                        for ci in cis:
                            ps_mm = psmm_pool.tile(
                                [128, C_TILE], f32, tag="psmm",
                                name=f"ps{blk}_{ci}",
                            )
                            pss[ci] = ps_mm
                        for db in range(N_DBLKS):
                            dsl = slice(db * 128, (db + 1) * 128)
                            for pi, (lhsT, hl) in enumerate(
                                passes(xth_t, xtl_t, dsl)
                            ):
                                rt = ct_tiles[db][hl]
                                for ci in cis:
                                    csl = slice(ci * C_TILE, (ci + 1) * C_TILE)
                                    nc.tensor.matmul(
                                        pss[ci][:], lhsT=lhsT, rhs=rt[:, csl],
                                        start=(db == 0 and pi == 0),
                                        stop=(db == N_DBLKS - 1 and pi == 2),
                                    )
                else:
                    for ci in range(N_CTILES):
                        csl = slice(ci * C_TILE, (ci + 1) * C_TILE)
                        ps_mm = psmm_pool.tile(
                            [128, C_TILE], f32, tag="psmm", name=f"ps{blk}_{ci}"
                        )
                        pss[ci] = ps_mm
                        k = 0
                        for db in range(N_DBLKS):
                            dsl = slice(db * 128, (db + 1) * 128)
                            for lhsT, hl in passes(xth_t, xtl_t, dsl):
                                nc.tensor.matmul(
                                    ps_mm[:], lhsT=lhsT,
                                    rhs=ct_tiles[db][hl][:, csl],
                                    start=(k == 0), stop=(k == 3 * N_DBLKS - 1),
                                )
                                k += 1

                # next block's transposes go ahead of this block's tail
                if blk + 1 < N_BLOCKS:
                    cur_prep = emit_xprep(blk + 1)

                # ---- psum drain + per-c-tile top-8 ----
                for ci in range(N_CTILES):
                    csl = slice(ci * C_TILE, (ci + 1) * C_TILE)
                    nc.vector.tensor_tensor(
                        out=score_t[:, csl], in0=pss[ci][:],
                        in1=sqc_sb[:, csl], op=Alu.subtract,
                    )
                    nc.vector.max(
                        max8s[:, ci * 8 : (ci + 1) * 8], score_t[:, csl]
                    )

                # ---- softmax / argmax tail ----
                max8 = small_pool.tile([128, 8], f32, tag="max8",
                                       name=f"max8_{blk}")
                idx8 = small_pool.tile([128, 8], u32, tag="idx8",
                                       name=f"idx8_{blk}")
                nc.vector.max(max8[:], max8s[:])
                nc.vector.max_index(idx8[:], max8[:], score_t[:])

                negmax = small_pool.tile([128, 1], f32, tag="negmax",
                                         name=f"negmax{blk}")
                nc.vector.tensor_scalar_mul(negmax[:], max8[:, 0:1], -1.0)

                sumexp = small_pool.tile([128, 1], f32, tag="sumexp",
                                         name=f"sumexp{blk}")
                nc.scalar.activation(
                    score_t[:], score_t[:], Act.Exp,
                    bias=negmax[:, 0:1], scale=1.0, accum_out=sumexp[:, 0:1],
                )
                rcp = small_pool.tile([128, 1], f32, tag="rcp", name=f"rcp{blk}")
                nc.vector.reciprocal(rcp[:], sumexp[:])
                # normalize + store in halves so the DMA overlaps the mul
                for h in range(2):
                    hsl = slice(h * (CB // 2), (h + 1) * (CB // 2))
                    nc.vector.tensor_scalar_mul(
                        score_t[:, hsl], score_t[:, hsl], rcp[:, 0:1]
                    )
                    nc.scalar.dma_start(
                        probs_out[row0 : row0 + 128, hsl], score_t[:, hsl]
                    )

                codes_t = small_pool.tile([128, 1], i32, tag="codes",
                                          name=f"codes{blk}")
                nc.vector.tensor_copy(codes_t[:], idx8[:, 0:1])
                nc.scalar.dma_start(codes_out[row0 : row0 + 128, :], codes_t[:])

                q_t = q_pool.tile([128, D], f32, tag="q", name=f"q{blk}")
                nc.gpsimd.indirect_dma_start(
                    out=q_t[:],
                    out_offset=None,
                    in_=cb_in[:, :],
                    in_offset=bass.IndirectOffsetOnAxis(ap=idx8[:, 0:1], axis=0),
                )
                # diff = q - x (in place in q); qst = x + diff (in place in x)
                nc.gpsimd.tensor_tensor(
                    out=q_t[:], in0=q_t[:], in1=x_t[:], op=Alu.subtract
                )
                nc.gpsimd.tensor_tensor(
                    out=x_t[:], in0=x_t[:], in1=q_t[:], op=Alu.add
                )
                nc.scalar.dma_start(qst_out[row0 : row0 + 128, :], x_t[:])
                nc.scalar.activation(
                    q_t[:], q_t[:], Act.Square,
                    accum_out=mse_cols[:, blk : blk + 1],
                )

            nc.sync.dma_start(mse_out[:], mse_cols[:])

    nc.compile()
    return nc


def _get_nc():
    if "nc" not in _compiled:
        _compiled["nc"] = _build_bass()
    return _compiled["nc"]


def kernel(slot_embeddings: np.ndarray, codebook: np.ndarray):
    import ml_dtypes
    from concourse.bass_utils import run_bass_kernel_spmd

    bf16 = ml_dtypes.bfloat16
    x = np.ascontiguousarray(slot_embeddings.reshape(N_TOTAL, D), dtype=np.float32)
    cb = np.ascontiguousarray(codebook, dtype=np.float32)

    xh = x.astype(bf16)
    xl = (x - xh.astype(np.float32)).astype(bf16)
    # pre-transposed per-core block layout [blk, p, db*128+n]:
    #   xt[core][blk, p, db, n] = xh.T[db*128+p, core*2048 + blk*128+n]
    def to_blocks(a):  # a: [N_TOTAL, D] bf16 -> [N_CORES, N_BLOCKS, 128, D]
        t = np.ascontiguousarray(a.T)  # [D, N_TOTAL]
        t = t.reshape(N_DBLKS, 128, N_CORES, N_BLOCKS, 128)
        return np.ascontiguousarray(t.transpose(2, 3, 1, 0, 4)).reshape(
            N_CORES, N_BLOCKS, 128, D
        )

    xth_b = to_blocks(xh)
    xtl_b = to_blocks(xl)
    ct2 = np.ascontiguousarray(cb.T) * np.float32(2.0)
    cth = ct2.astype(bf16)
    ctl = (ct2 - cth.astype(np.float32)).astype(bf16)
    sqc = np.sum(cb.astype(np.float64) ** 2, axis=1).astype(np.float32)
    sqc_rep = np.ascontiguousarray(np.broadcast_to(sqc[None, :], (128, CB)))

    nc = _get_nc()
    in_maps = []
    for c in range(N_CORES):
        sl = slice(c * N_CORE, (c + 1) * N_CORE)
        in_maps.append(
            {
                "x": x[sl],
                "xth": xth_b[c],
                "xtl": xtl_b[c],
                "cth": cth,
                "ctl": ctl,
                "sqc": sqc_rep,
                "cb": cb,
            }
        )
    res = run_bass_kernel_spmd(nc, in_maps, core_ids=list(range(N_CORES)))
    _compiled["last_res"] = res

    probs = np.empty((N_TOTAL, CB), dtype=np.float32)
    qst = np.empty((N_TOTAL, D), dtype=np.float32)
    codes = np.empty((N_TOTAL,), dtype=np.int32)
    sum_sq = np.float32(0.0)
    for c, r in enumerate(res.results):
        sl = slice(c * N_CORE, (c + 1) * N_CORE)
        probs[sl] = r["probs"]
        qst[sl] = r["qst"]
        codes[sl] = r["codes"][:, 0]
        sum_sq = np.float32(sum_sq + np.float32(np.sum(r["msepart"], dtype=np.float64)))

    mse = np.float32(sum_sq / np.float32(N_TOTAL * D))
    vq_loss = np.float32(mse + np.float32(BETA) * mse)

    return (
        qst.reshape(BATCH, K, D),
        codes.reshape(BATCH, K),
        probs.reshape(BATCH, K, CB),
        vq_loss,
    )


# revision 20
# speedup vs baseline: 1.0845x; 1.0002x over previous
"""VQ codebook (DiscreteBottleneck) Trainium2 kernel.

Problem: slot_embeddings [64, 256, 1024] f32, codebook [4096, 1024] f32.
Returns (quantized_st [64,256,1024] f32, codes [64,256] i32,
         probs [64,256,4096] f32, vq_loss f32 scalar).

Strategy: data-parallel over batch across 8 NeuronCores (2048 flat rows
per core). Per core:
  score[n, c] = 2 * x_n . c_c - ||c_c||^2   (= -squared-distance + ||x||^2)
  codes = argmax_c score        (same as argmin of distance)
  probs = softmax(score)        (softmax is invariant to the per-row shift)
  quantized = codebook[codes]   (indirect-DMA row gather)
  quantized_st = x + (q - x);  vq_loss partial = sum((q - x)^2)

The 2048x4096x1024 contraction runs on the PE as a 3-pass split-bf16
matmul (x = xh + xl, c = ch + cl in bf16; xc ~= xh.ch + xh.cl + xl.ch).
Products of bf16 pairs are exact in the fp32 PSUM accumulator, so this
matches fp32-matmul argmin decisions (verified == fp64 argmin on the
problem's inputs) at 3 PE cycles/column instead of fp32's 4.
Host pre-computes the bf16 hi/lo splits of x and of 2*C^T, plus ||c||^2
replicated across partitions; host does the final loss reduction (the
"all-reduce").

The host also pre-transposes the x splits into a DMA-friendly per-block
layout ([blk, p=d-within-dblock, db*128+n]), so the device does no
transposes at all and all 8 PSUM banks go to matmul accumulation.
Pipelining: block b+1's x DMAs are emitted right after block b's matmuls
(ahead of block b's softmax tail in engine priority).  Block 0
accumulates d-block-outer across all 8 c-tiles so its first matmuls only
wait on the first codebook slice instead of the whole 16.8MB load.
"""

import numpy as np

N_CORES = 8
BATCH, K, D = 64, 256, 1024
CB = 4096
N_TOTAL = BATCH * K          # 16384
N_CORE = N_TOTAL // N_CORES  # 2048
N_BLOCKS = N_CORE // 128     # 16
C_TILE = 512
N_CTILES = CB // C_TILE      # 8
N_DBLKS = D // 128           # 8
BETA = 0.25

_compiled = {}


def _build_bass():
    import concourse.bass as bass
    import concourse.bacc as bacc
    import concourse.mybir as mybir
    import concourse.tile as tile
    from concourse.masks import make_identity

    f32 = mybir.dt.float32
    bf16 = mybir.dt.bfloat16
    u32 = mybir.dt.uint32
    i32 = mybir.dt.int32
    Alu = mybir.AluOpType
    Act = mybir.ActivationFunctionType

    nc = bacc.Bacc("TRN2", target_bir_lowering=False, debug=False)
    x_in = nc.dram_tensor("x", [N_CORE, D], f32, kind="ExternalInput")
    # pre-transposed per-block x splits: [blk][p=d-in-dblk][db*128+n]
    xth_in = nc.dram_tensor("xth", [N_BLOCKS, 128, D], bf16, kind="ExternalInput")
    xtl_in = nc.dram_tensor("xtl", [N_BLOCKS, 128, D], bf16, kind="ExternalInput")
    cth_in = nc.dram_tensor("cth", [D, CB], bf16, kind="ExternalInput")  # hi(2*C^T)
    ctl_in = nc.dram_tensor("ctl", [D, CB], bf16, kind="ExternalInput")  # lo(2*C^T)
    sqc_in = nc.dram_tensor("sqc", [128, CB], f32, kind="ExternalInput")
    cb_in = nc.dram_tensor("cb", [CB, D], f32, kind="ExternalInput")
    probs_out = nc.dram_tensor("probs", [N_CORE, CB], f32, kind="ExternalOutput")
    qst_out = nc.dram_tensor("qst", [N_CORE, D], f32, kind="ExternalOutput")
    codes_out = nc.dram_tensor("codes", [N_CORE, 1], i32, kind="ExternalOutput")
    mse_out = nc.dram_tensor("msepart", [128, N_BLOCKS], f32, kind="ExternalOutput")

    cth_view = cth_in.rearrange("(t p) c -> t p c", p=128)
    ctl_view = ctl_in.rearrange("(t p) c -> t p c", p=128)

    with tile.TileContext(nc) as tc:
        with (
            tc.tile_pool(name="const", bufs=1) as const_pool,
            tc.tile_pool(name="ct", bufs=1) as ct_pool,
            tc.tile_pool(name="score", bufs=2) as score_pool,
            tc.tile_pool(name="x", bufs=2) as x_pool,
            tc.tile_pool(name="xt", bufs=2) as xt_pool,
            tc.tile_pool(name="q", bufs=2) as q_pool,
            tc.tile_pool(name="small", bufs=2) as small_pool,
            tc.tile_pool(name="acc", bufs=1) as acc_pool,
            tc.tile_pool(name="psmm", bufs=8, space="PSUM") as psmm_pool,
        ):
            def emit_xprep(nb):
                """DMA block nb's pre-transposed x splits + x rows."""
                row0 = nb * 128
                xth_t = xt_pool.tile([128, D], bf16, tag="xth", name=f"xth{nb}")
                nc.sync.dma_start(xth_t[:], xth_in[nb])
                xtl_t = xt_pool.tile([128, D], bf16, tag="xtl", name=f"xtl{nb}")
                nc.sync.dma_start(xtl_t[:], xtl_in[nb])
                x_t = x_pool.tile([128, D], f32, tag="x", name=f"x{nb}")
                nc.scalar.dma_start(x_t[:], x_in[row0 : row0 + 128, :])
                return xth_t, xtl_t, x_t

            # block 0's x rows first in the DMA queue, then the codebook
            cur_prep = emit_xprep(0)

            ct_tiles = []  # [(hi, lo)] per d-block
            half = CB // 2
            for t in range(N_DBLKS):
                cth_t = ct_pool.tile([128, CB], bf16, tag=f"cth{t}")
                ctl_t = ct_pool.tile([128, CB], bf16, tag=f"ctl{t}")
                # half-column DMAs in block-0 consumption order so the
                # first matmuls unblock at 1MB granularity
                for h in range(2):
                    hs = slice(h * half, (h + 1) * half)
                    nc.sync.dma_start(cth_t[:, hs], cth_view[t][:, hs])
                    nc.sync.dma_start(ctl_t[:, hs], ctl_view[t][:, hs])
                ct_tiles.append((cth_t, ctl_t))

            sqc_sb = const_pool.tile([128, CB], f32, tag="sqc")
            nc.scalar.dma_start(sqc_sb[:], sqc_in[:])

            mse_cols = acc_pool.tile([128, N_BLOCKS], f32, tag="msecols")

            passes = lambda xth_t, xtl_t, dsl: (
                (xth_t[:, dsl], 0), (xth_t[:, dsl], 1), (xtl_t[:, dsl], 0)
            )

            for blk in range(N_BLOCKS):
                row0 = blk * 128
                xth_t, xtl_t, x_t = cur_prep

                score_t = score_pool.tile([128, CB], f32, tag="score",
                                          name=f"score{blk}")
                max8s = small_pool.tile([128, 8 * N_CTILES], f32, tag="max8s",
                                        name=f"max8s{blk}")

                # ---- matmuls ----
                pss = {}
                if blk == 0:
                    # d-block-outer over all 8 c-tiles, split by ct column
                    # half so each matmul group unblocks on a 1MB arrival
                    for ci in range(N_CTILES):
                        ps_mm = psmm_pool.tile(
                            [128, C_TILE], f32, tag="psmm",
                            name=f"ps{blk}_{ci}",
                        )
                        pss[ci] = ps_mm
                    for db in range(N_DBLKS):
                        dsl = slice(db * 128, (db + 1) * 128)
                        for h in range(2):
                            for pi, (lhsT, hl) in enumerate(
                                passes(xth_t, xtl_t, dsl)
                            ):
                                rt = ct_tiles[db][hl]
                                for ci in range(h * 4, h * 4 + 4):
                                    csl = slice(ci * C_TILE, (ci + 1) * C_TILE)
                                    nc.tensor.matmul(
                                        pss[ci][:], lhsT=lhsT, rhs=rt[:, csl],
                                        start=(db == 0 and pi == 0),
                                        stop=(db == N_DBLKS - 1 and pi == 2),
                                    )
                else:
                    for ci in range(N_CTILES):
                        csl = slice(ci * C_TILE, (ci + 1) * C_TILE)
                        ps_mm = psmm_pool.tile(
                            [128, C_TILE], f32, tag="psmm", name=f"ps{blk}_{ci}"
                        )
                        pss[ci] = ps_mm
                        k = 0
                        for db in range(N_DBLKS):
                            dsl = slice(db * 128, (db + 1) * 128)
                            for lhsT, hl in passes(xth_t, xtl_t, dsl):
                                nc.tensor.matmul(
                                    ps_mm[:], lhsT=lhsT,
                                    rhs=ct_tiles[db][hl][:, csl],
                                    start=(k == 0), stop=(k == 3 * N_DBLKS - 1),
                                )
                                k += 1

                # next block's transposes go ahead of this block's tail
                if blk + 1 < N_BLOCKS:
                    cur_prep = emit_xprep(blk + 1)

                # ---- psum drain + per-c-tile top-8 ----
                for ci in range(N_CTILES):
                    csl = slice(ci * C_TILE, (ci + 1) * C_TILE)
                    nc.vector.tensor_tensor(
                        out=score_t[:, csl], in0=pss[ci][:],
                        in1=sqc_sb[:, csl], op=Alu.subtract,
                    )
                    nc.vector.max(
                        max8s[:, ci * 8 : (ci + 1) * 8], score_t[:, csl]
                    )

                # ---- softmax / argmax tail ----
                max8 = small_pool.tile([128, 8], f32, tag="max8",
                                       name=f"max8_{blk}")
                idx8 = small_pool.tile([128, 8], u32, tag="idx8",
                                       name=f"idx8_{blk}")
                nc.vector.max(max8[:], max8s[:])
                nc.vector.max_index(idx8[:], max8[:], score_t[:])

                negmax = small_pool.tile([128, 1], f32, tag="negmax",
                                         name=f"negmax{blk}")
                nc.vector.tensor_scalar_mul(negmax[:], max8[:, 0:1], -1.0)

                sumexp = small_pool.tile([128, 1], f32, tag="sumexp",
                                         name=f"sumexp{blk}")
                nc.scalar.activation(
                    score_t[:], score_t[:], Act.Exp,
                    bias=negmax[:, 0:1], scale=1.0, accum_out=sumexp[:, 0:1],
                )
                rcp = small_pool.tile([128, 1], f32, tag="rcp", name=f"rcp{blk}")
                nc.vector.reciprocal(rcp[:], sumexp[:])
                # normalize + store in halves so the DMA overlaps the mul
                for h in range(2):
                    hsl = slice(h * (CB // 2), (h + 1) * (CB // 2))
                    nc.vector.tensor_scalar_mul(
                        score_t[:, hsl], score_t[:, hsl], rcp[:, 0:1]
                    )
                    nc.scalar.dma_start(
                        probs_out[row0 : row0 + 128, hsl], score_t[:, hsl]
                    )

                codes_t = small_pool.tile([128, 1], i32, tag="codes",
                                          name=f"codes{blk}")
                nc.vector.tensor_copy(codes_t[:], idx8[:, 0:1])
                nc.scalar.dma_start(codes_out[row0 : row0 + 128, :], codes_t[:])

                q_t = q_pool.tile([128, D], f32, tag="q", name=f"q{blk}")
                nc.gpsimd.indirect_dma_start(
                    out=q_t[:],
                    out_offset=None,
                    in_=cb_in[:, :],
                    in_offset=bass.IndirectOffsetOnAxis(ap=idx8[:, 0:1], axis=0),
                )
                # diff = q - x (in place in q); qst = x + diff (in place in x)
                nc.gpsimd.tensor_tensor(
                    out=q_t[:], in0=q_t[:], in1=x_t[:], op=Alu.subtract
                )
                nc.gpsimd.tensor_tensor(
                    out=x_t[:], in0=x_t[:], in1=q_t[:], op=Alu.add
                )
                nc.scalar.dma_start(qst_out[row0 : row0 + 128, :], x_t[:])
                nc.scalar.activation(
                    q_t[:], q_t[:], Act.Square,
                    accum_out=mse_cols[:, blk : blk + 1],
                )

            nc.sync.dma_start(mse_out[:], mse_cols[:])

    nc.compile()
    return nc


def _get_nc():
    if "nc" not in _compiled:
        _compiled["nc"] = _build_bass()
    return _compiled["nc"]


def kernel(slot_embeddings: np.ndarray, codebook: np.ndarray):
    import ml_dtypes
    from concourse.bass_utils import run_bass_kernel_spmd

    bf16 = ml_dtypes.bfloat16
    x = np.ascontiguousarray(slot_embeddings.reshape(N_TOTAL, D), dtype=np.float32)
    cb = np.ascontiguousarray(codebook, dtype=np.float32)

    xh = x.astype(bf16)
    xl = (x - xh.astype(np.float32)).astype(bf16)
    # pre-transposed per-core block layout [blk, p, db*128+n]:
    #   xt[core][blk, p, db, n] = xh.T[db*128+p, core*2048 + blk*128+n]
    def to_blocks(a):  # a: [N_TOTAL, D] bf16 -> [N_CORES, N_BLOCKS, 128, D]
        t = np.ascontiguousarray(a.T)  # [D, N_TOTAL]
        t = t.reshape(N_DBLKS, 128, N_CORES, N_BLOCKS, 128)
        return np.ascontiguousarray(t.transpose(2, 3, 1, 0, 4)).reshape(
            N_CORES, N_BLOCKS, 128, D
        )

    xth_b = to_blocks(xh)
    xtl_b = to_blocks(xl)
    ct2 = np.ascontiguousarray(cb.T) * np.float32(2.0)
    cth = ct2.astype(bf16)
    ctl = (ct2 - cth.astype(np.float32)).astype(bf16)
    sqc = np.sum(cb.astype(np.float64) ** 2, axis=1).astype(np.float32)
    sqc_rep = np.ascontiguousarray(np.broadcast_to(sqc[None, :], (128, CB)))

    nc = _get_nc()
    in_maps = []
    for c in range(N_CORES):
        sl = slice(c * N_CORE, (c + 1) * N_CORE)
        in_maps.append(
            {
                "x": x[sl],
                "xth": xth_b[c],
                "xtl": xtl_b[c],
                "cth": cth,
                "ctl": ctl,
                "sqc": sqc_rep,
                "cb": cb,
            }
        )
    res = run_bass_kernel_spmd(nc, in_maps, core_ids=list(range(N_CORES)))
    _compiled["last_res"] = res

    probs = np.empty((N_TOTAL, CB), dtype=np.float32)
    qst = np.empty((N_TOTAL, D), dtype=np.float32)
    codes = np.empty((N_TOTAL,), dtype=np.int32)
    sum_sq = np.float32(0.0)
    for c, r in enumerate(res.results):
        sl = slice(c * N_CORE, (c + 1) * N_CORE)
        probs[sl] = r["probs"]
        qst[sl] = r["qst"]
        codes[sl] = r["codes"][:, 0]
        sum_sq = np.float32(sum_sq + np.float32(np.sum(r["msepart"], dtype=np.float64)))

    mse = np.float32(sum_sq / np.float32(N_TOTAL * D))
    vq_loss = np.float32(mse + np.float32(BETA) * mse)

    return (
        qst.reshape(BATCH, K, D),
        codes.reshape(BATCH, K),
        probs.reshape(BATCH, K, CB),
        vq_loss,
    )
